# revision 7
# baseline (speedup 1.0000x reference)
"""Trainium2 Bass kernel for AdaptiveStochasticSNN (v3: tapered windows, packed
PSUM, per-block FC2, ACT-engine bias-subtract, incremental reduce).

Model: x[B,T,D] -> FC1(D->H) -> StochasticAdaptiveLIF -> FC2(H->A)
       -> StochasticAdaptiveLIF -> mean spikes over T.   B,T,D,H,A = 256,64,6400,1000,4

Strategy (8 NeuronCores, data-parallel over batch, 32 batches/core):
- FC1 in fp16 at the full 2.4GHz PE column rate; W1 (13.1MB fp16) SBUF-resident.
- The bernoulli draw  u < sigmoid(mem - 1 - theta)  is host-transformed to
  logit(u) + 1 + theta < mem; theta tracked as psi = 20*theta - 10 so its
  update is one DVE op. lu1 stored fp16 (mixed-dtype DVE ops).
- Time processed in 8 GEMM windows of [16,8,8,8,8,8,4,4] steps. LIF blocks are
  [8,8,8,8,8,8,8,4,4] (w0 = 2 blocks). Per window w: GEMM(w) overlaps
  LIF1(blocks of w-1) on DVE; FC2(b) matmuls run right after GEMM(w) for w-1's
  blocks (LIF1 always finishes first -> no PE stall); c2 = rs2 - ps2 runs on
  the ACT engine (bias=rs2, scale=-1); LIF2(w-1) + partial spike reduction on
  DVE behind LIF1(w).
- PSUM: one FIFO tag, bufs=8 banks. w0 uses 8 full banks; later windows pack
  2 h-chunks per bank ([128, 2, ncol]) so 4 banks/window + fc2 banks rotate.
- Startup: first DMAs are single-kc slivers (x kc0, W1 kc0) so the first
  matmul issues ~2us after the preamble barrier.
"""

import sys

sys.path.insert(0, "/opt/trn_rl_repo")

import numpy as np

# ---- problem dims (hardcoded; kernel.py must be self-contained) ----
B, T, D, H, A = 256, 64, 6400, 1000, 4
HP = 1024          # H padded to 8*128
NCORES = 8
BC = B // NCORES   # 32 batches per core
BT = BC * T        # 2048 bt-columns per core, ordered bt = t*BC + b
KC = D // 128      # 50 contraction chunks
MC = HP // 128     # 8 h-chunks
WSTEPS = [16, 8, 8, 8, 8, 8, 4, 4]      # GEMM window timesteps
NWIN = len(WSTEPS)
WCOLS = [s * BC for s in WSTEPS]
WCOL0 = [0]
for _c in WCOLS:
    WCOL0.append(WCOL0[-1] + _c)
WSTEP0 = [0]
for _s in WSTEPS:
    WSTEP0.append(WSTEP0[-1] + _s)
# LIF blocks: window 0 split into two 8-step blocks; otherwise block == window
BSTEPS = [8, 8] + WSTEPS[1:]            # [8,8,8,8,8,8,8,4,4]
NBLK = len(BSTEPS)
BSTEP0 = [0]
for _s in BSTEPS:
    BSTEP0.append(BSTEP0[-1] + _s)
BCOLS = [s * BC for s in BSTEPS]
BCOL0 = [0]
for _c in BCOLS:
    BCOL0.append(BCOL0[-1] + _c)
BLOCKS_OF_W = [[0, 1]] + [[w + 1] for w in range(1, NWIN)]
BETA = 0.9
TH_DEC = 0.9
TH_PLUS = 0.05

_CACHE = {}


def _build_graph():
    import concourse.bass as bass
    import concourse.tile as tile
    from concourse import bacc, mybir
    from concourse.alu_op_type import AluOpType as op
    from contextlib import ExitStack

    F32 = mybir.dt.float32
    F16 = mybir.dt.float16
    AF = mybir.ActivationFunctionType

    nc = bacc.Bacc("TRN2", target_bir_lowering=False, debug=False, num_devices=NCORES)

    # host-prepped layouts, all contiguous per partition:
    # xh:  [p, w, kc, col]   fp16 (col within window)
    # w1h: [p, kc, mc, m]    fp16
    # lu1: [p, blk, mc, c]   fp16 (c within block)
    xh = nc.declare_dram_parameter("xh", [128, KC * BT], F16, isOutput=False)
    w1h = nc.declare_dram_parameter("w1h", [128, KC * MC * 128], F16, isOutput=False)
    b1t = nc.declare_dram_parameter("b1t", [128, MC], F32, isOutput=False)
    lu1 = nc.declare_dram_parameter("lu1", [128, MC * BT], F16, isOutput=False)
    lu2 = nc.declare_dram_parameter("lu2", [A, T * BC], F32, isOutput=False)
    w2t = nc.declare_dram_parameter("w2t", [128, MC, A], F16, isOutput=False)
    rs2c = nc.declare_dram_parameter("rs2c", [A, 512], F32, isOutput=False)
    out = nc.declare_dram_parameter("out", [A, BC], F32, isOutput=True)

    with tile.TileContext(nc) as tc, ExitStack() as ctx:
        p_w1 = ctx.enter_context(tc.tile_pool(name="w1p", bufs=1))
        p_x = ctx.enter_context(tc.tile_pool(name="xp", bufs=2))
        p_cur = ctx.enter_context(tc.tile_pool(name="curp", bufs=3))
        p_lu = ctx.enter_context(tc.tile_pool(name="lup", bufs=2))
        p_ge = ctx.enter_context(tc.tile_pool(name="gep", bufs=2))
        p_st = ctx.enter_context(tc.tile_pool(name="stp", bufs=1))
        p_sc = ctx.enter_context(tc.tile_pool(name="scp", bufs=2))
        p_ps = ctx.enter_context(
            tc.tile_pool(name="psp", bufs=8, space=bass.MemorySpace.PSUM)
        )

        # ---- constants / states ----
        b1_sb = p_st.tile([128, MC], F32, name="b1_sb")
        nc.sync.dma_start(b1_sb[:], b1t[:])
        w2_sb = p_st.tile([128, MC, A], F16, name="w2_sb")
        nc.sync.dma_start(w2_sb[:], w2t[:])
        rs2_sb = p_st.tile([A, 512], F32, name="rs2_sb")
        nc.sync.dma_start(rs2_sb[:], rs2c[:])
        lu2_sb = p_st.tile([A, T * BC], F32, name="lu2_sb")
        nc.sync.dma_start(lu2_sb[:], lu2[:])

        w1_sb = p_w1.tile([128, KC * MC * 128], F16, name="w1_sb")

        mem = p_st.tile([128, MC, BC], F32, name="mem")
        nc.gpsimd.memset(mem[:], 0.0)
        psi = p_st.tile([128, MC, BC], F32, name="psi")
        nc.gpsimd.memset(psi[:], -10.0)
        mem2 = p_st.tile([A, BC], F32, name="mem2")
        nc.gpsimd.memset(mem2[:], 0.0)
        psi2 = p_st.tile([A, BC], F32, name="psi2")
        nc.gpsimd.memset(psi2[:], -10.0)
        sum2g = p_st.tile([A, BC], F32, name="sum2g")
        nc.gpsimd.memset(sum2g[:], 0.0)

        cur_tiles = [None] * NBLK
        ge_tiles = [None] * NBLK
        lu_tiles = [None] * NBLK
        c2_tiles = [None] * NBLK
        ge2_tiles = [None] * NBLK
        ps2_tiles = [None] * NBLK

        def emit_lu_dma(b):
            bs = BSTEPS[b]
            lu_t = p_lu.tile([128, MC, bs * BC], F16, tag="lu", name=f"lu_{b}")
            lu_tiles[b] = lu_t
            src = BCOL0[b] * MC
            nc.sync.dma_start(lu_t[:], lu1[:, src : src + MC * bs * BC])

        def emit_lif1(b):
            """LIF1 recurrence for block b on DVE."""
            bs = BSTEPS[b]
            cur1 = cur_tiles[b]
            lu_t = lu_tiles[b]
            ge_t = p_ge.tile([128, MC, bs * BC], F16, tag="ge", name=f"ge_{b}")
            ge_tiles[b] = ge_t
            for s in range(bs):
                c_sl = cur1[:, :, s * BC : (s + 1) * BC]
                mi = p_sc.tile([128, MC, BC], F32, tag="mi", name=f"mi_{b}_{s}")
                nc.vector.scalar_tensor_tensor(
                    mi[:], mem[:], BETA, c_sl, op0=op.mult, op1=op.add
                )
                lp = p_sc.tile([128, MC, BC], F32, tag="lp", name=f"lp_{b}_{s}")
                lu_sl = lu_t[:, :, s * BC : (s + 1) * BC]
                nc.vector.scalar_tensor_tensor(
                    lp[:], psi[:], TH_PLUS, lu_sl, op0=op.mult, op1=op.add
                )
                ge_sl = ge_t[:, :, s * BC : (s + 1) * BC]
                nc.vector.tensor_tensor(ge_sl, mi[:], lp[:], op.is_le)
                nc.vector.tensor_tensor(mem[:], mi[:], ge_sl, op.mult)
                nc.vector.scalar_tensor_tensor(
                    psi[:], psi[:], TH_DEC, ge_sl, op0=op.mult, op1=op.subtract
                )

        def emit_fc2_mm(b):
            """FC2 matmuls for block b: ps2[A, bcol] = W2 @ ge_b (PE)."""
            bcol = BCOLS[b]
            ge_t = ge_tiles[b]
            ps2 = p_ps.tile([A, bcol], F32, tag="bank", name=f"ps2_{b}")
            ps2_tiles[b] = ps2
            for k2 in range(MC):
                nc.tensor.matmul(
                    ps2[:],
                    w2_sb[:, k2, :],
                    ge_t[:, k2, :],
                    start=(k2 == 0),
                    stop=(k2 == MC - 1),
                )

        def emit_fc2_sub(b):
            """c2 = rs2 - ps2 on the ACT engine (bias=rs2 per-partition)."""
            bcol = BCOLS[b]
            c2 = p_sc.tile([A, bcol], F32, tag="c2", name=f"c2_{b}")
            nc.scalar.activation(
                c2[:], ps2_tiles[b][:], AF.Identity, bias=rs2_sb[:, 0:1], scale=-1.0
            )
            c2_tiles[b] = c2

        def emit_lif2(b):
            """LIF2 recurrence for block b on DVE; ge2 stored [A, BC, bs]."""
            bs = BSTEPS[b]
            ge2_t = p_sc.tile([A, BC, bs], F16, tag="ge2", name=f"ge2_{b}")
            ge2_tiles[b] = ge2_t
            c2 = c2_tiles[b]
            for s in range(bs):
                t = BSTEP0[b] + s
                cur2 = c2[:, s * BC : (s + 1) * BC]
                m2i = p_sc.tile([A, BC], F32, tag="m2i", name=f"m2i_{t}")
                nc.vector.scalar_tensor_tensor(
                    m2i[:], mem2[:], BETA, cur2, op0=op.mult, op1=op.add
                )
                lp2 = p_sc.tile([A, BC], F32, tag="lp2", name=f"lp2_{t}")
                nc.vector.scalar_tensor_tensor(
                    lp2[:],
                    psi2[:],
                    TH_PLUS,
                    lu2_sb[:, t * BC : (t + 1) * BC],
                    op0=op.mult,
                    op1=op.add,
                )
                ge2_sl = ge2_t[:, :, s]
                nc.vector.tensor_tensor(ge2_sl, m2i[:], lp2[:], op.is_le)
                nc.vector.tensor_tensor(mem2[:], m2i[:], ge2_sl, op.mult)
                nc.vector.scalar_tensor_tensor(
                    psi2[:], psi2[:], TH_DEC, ge2_sl, op0=op.mult, op1=op.subtract
                )

        def emit_reduce(b):
            """Accumulate spike counts for block b into sum2g (DVE)."""
            tmp = p_sc.tile([A, BC], F32, tag="rtmp", name=f"rtmp_{b}")
            nc.vector.tensor_reduce(
                tmp[:], ge2_tiles[b][:], mybir.AxisListType.X, op.add
            )
            nc.vector.tensor_tensor(sum2g[:], sum2g[:], tmp[:], op.add)

        # ---------------- main window loop ----------------
        for w in range(NWIN):
            ncol = WCOLS[w]
            blocks = BLOCKS_OF_W[w]
            # cur tiles for this window's blocks
            for b in blocks:
                cur_tiles[b] = p_cur.tile(
                    [128, MC, BCOLS[b]], F32, tag="cur", name=f"cur_{b}"
                )
            # psum accumulators: one full bank per h-chunk (two accumulation
            # groups must NOT share a bank: interleaved start/accumulate to
            # one bank corrupts the other group's region)
            accs = [
                p_ps.tile([128, ncol], F32, tag="bank", name=f"acc_{w}_{mc}")
                for mc in range(MC)
            ]

            def accsl(mc):
                return accs[mc][:]

            # x DMA groups + matmuls
            if w == 0:
                XG = 5
                groups = [list(range(g * XG, (g + 1) * XG)) for g in range(KC // XG)]
            else:
                XG = 25
                groups = [list(range(g * XG, (g + 1) * XG)) for g in range(KC // XG)]
            for gi, kcs in enumerate(groups):
                xg = p_x.tile(
                    [128, len(kcs) * ncol],
                    F16,
                    tag=("x5" if w == 0 else "x25"),
                    name=f"x_{w}_{gi}",
                )
                src = WCOL0[w] * KC + kcs[0] * ncol
                if w == 0 and gi == 0:
                    # sliver DMAs: first matmul waits on only x kc0 + W1 kc0
                    nc.sync.dma_start(xg[:, :ncol], xh[:, src : src + ncol])
                    nc.sync.dma_start(
                        w1_sb[:, : MC * 128], w1h[:, : MC * 128]
                    )
                    nc.sync.dma_start(
                        xg[:, ncol:], xh[:, src + ncol : src + XG * ncol]
                    )
                    nc.sync.dma_start(
                        w1_sb[:, MC * 128 : XG * MC * 128],
                        w1h[:, MC * 128 : XG * MC * 128],
                    )
                else:
                    nc.sync.dma_start(xg[:], xh[:, src : src + len(kcs) * ncol])
                    if w == 0:
                        wsrc = gi * XG * MC * 128
                        nc.sync.dma_start(
                            w1_sb[:, wsrc : wsrc + XG * MC * 128],
                            w1h[:, wsrc : wsrc + XG * MC * 128],
                        )
                # lu prefetch for this window's blocks, spread mid-window
                if w == 0 and gi == 6:
                    emit_lu_dma(0)
                    emit_lu_dma(1)
                elif w > 0 and gi == 1:
                    emit_lu_dma(w + 1)
                for i, kc in enumerate(kcs):
                    for mc in range(MC):
                        nc.tensor.matmul(
                            accsl(mc),
                            w1_sb[
                                :, (kc * MC + mc) * 128 : (kc * MC + mc + 1) * 128
                            ],
                            xg[:, i * ncol : (i + 1) * ncol],
                            start=(kc == 0),
                            stop=(kc == KC - 1),
                        )
                        if kc == KC - 1:
                            # psum -> sbuf (+b1), freeing the bank early
                            for bi, b in enumerate(blocks):
                                nc.scalar.activation(
                                    cur_tiles[b][:, mc, :],
                                    accs[mc][:, bi * 256 : bi * 256 + BCOLS[b]],
                                    AF.Identity,
                                    bias=b1_sb[:, mc : mc + 1],
                                    scale=1.0,
                                )
            # PE: FC2 for previous window's blocks (LIF1 finished during this
            # window); ACT: c2 subs right behind
            if w >= 1:
                for b in BLOCKS_OF_W[w - 1]:
                    emit_fc2_mm(b)
                for b in BLOCKS_OF_W[w - 1]:
                    emit_fc2_sub(b)
            # DVE: LIF1 for this window's blocks, then LIF2 + reduce for the
            # previous window's blocks
            for b in blocks:
                emit_lif1(b)
            if w >= 1:
                for b in BLOCKS_OF_W[w - 1]:
                    emit_lif2(b)
                    emit_reduce(b)

        # ---------- tail ----------
        bl = NBLK - 1
        emit_fc2_mm(bl)
        emit_fc2_sub(bl)
        emit_lif2(bl)
        emit_reduce(bl)

        outf = p_st.tile([A, BC], F32, name="outf")
        nc.scalar.activation(outf[:], sum2g[:], AF.Copy, bias=1.0, scale=-1.0 / T)
        nc.sync.dma_start(out[:], outf[:])

    nc.compile()
    return nc


def _host_prep(x, W1, b1, W2, b2, u1, u2):
    """Shard + lay out inputs for the 8 cores. Returns in_maps."""
    x = np.asarray(x, dtype=np.float32)
    W1 = np.asarray(W1, dtype=np.float32)
    b1 = np.asarray(b1, dtype=np.float32)
    W2 = np.asarray(W2, dtype=np.float32)
    b2 = np.asarray(b2, dtype=np.float32)

    BIG = np.float32(30000.0)
    with np.errstate(divide="ignore"):
        u1d = np.asarray(u1, dtype=np.float64)
        lu1f = np.clip(np.log(u1d / (1.0 - u1d)) + 1.5, -3e4, 3e4).astype(np.float32)
        u2d = np.asarray(u2, dtype=np.float64)
        lu2f = np.clip(np.log(u2d / (1.0 - u2d)) + 1.5, -1e30, 1e30).astype(np.float32)

    W1TP = np.zeros((D, HP), np.float32)
    W1TP[:, :H] = W1.T
    w1h = np.ascontiguousarray(
        W1TP.reshape(KC, 128, MC, 128).transpose(1, 0, 2, 3).reshape(128, KC * MC * 128)
    ).astype(np.float16)

    b1p = np.zeros((HP,), np.float32)
    b1p[:H] = b1
    b1t = np.ascontiguousarray(b1p.reshape(MC, 128).T)  # [128, MC]

    W2f16 = W2.T.astype(np.float16)  # [H, A]
    W2TP = np.zeros((HP, A), np.float16)
    W2TP[:H, :] = W2f16
    w2t = np.ascontiguousarray(W2TP.reshape(MC, 128, A).transpose(1, 0, 2))

    rs2 = (W2f16.astype(np.float64).sum(axis=0) + b2).astype(np.float32)  # [A]
    rs2c = np.ascontiguousarray(np.repeat(rs2[:, None], 512, axis=1))  # [A, 512]

    in_maps = []
    for c in range(NCORES):
        bs, be = c * BC, (c + 1) * BC
        # x: [D, bt] -> [p, w, kc, col] fp16
        xt = x[bs:be].transpose(2, 1, 0).reshape(D, BT).astype(np.float16)
        arr = xt.reshape(KC, 128, BT).transpose(1, 0, 2)  # [p, kc, bt]
        xh_c = np.concatenate(
            [
                np.ascontiguousarray(arr[:, :, WCOL0[w] : WCOL0[w + 1]]).reshape(
                    128, -1
                )
                for w in range(NWIN)
            ],
            axis=1,
        )
        xh_c = np.ascontiguousarray(xh_c)
        # lu1: [p, blk, mc, c] fp16
        lu_c = np.full((T, BC, HP), BIG, np.float32)
        lu_c[:, :, :H] = lu1f[:, bs:be, :]
        lu_c = lu_c.transpose(2, 0, 1).reshape(HP, BT)  # [h, t*BC+b]
        lu_c = lu_c.reshape(MC, 128, BT).transpose(1, 0, 2)  # [p, mc, bt]
        lu_b = np.concatenate(
            [
                np.ascontiguousarray(
                    lu_c[:, :, BCOL0[b] : BCOL0[b + 1]]
                ).reshape(128, -1)
                for b in range(NBLK)
            ],
            axis=1,
        ).astype(np.float16)
        lu_b = np.ascontiguousarray(lu_b)
        # lu2: [A, T*BC]
        lu2_c = np.ascontiguousarray(
            lu2f[:, bs:be, :].transpose(2, 0, 1).reshape(A, T * BC)
        )
        in_maps.append(
            {
                "xh": xh_c,
                "w1h": w1h,
                "b1t": b1t,
                "lu1": lu_b,
                "lu2": lu2_c,
                "w2t": w2t,
                "rs2c": rs2c,
            }
        )
    return in_maps


def run(inputs, trace=False):
    """Build (cached), run on 8 cores, gather. Returns (out, BassKernelResults)."""
    from concourse.bass_utils import run_bass_kernel_spmd

    if "nc" not in _CACHE:
        _CACHE["nc"] = _build_graph()
    nc = _CACHE["nc"]
    in_maps = _host_prep(**inputs)
    res = run_bass_kernel_spmd(nc, in_maps, core_ids=list(range(NCORES)), trace=trace)
    out = np.concatenate(
        [res.results[c]["out"].T for c in range(NCORES)], axis=0
    )
    return np.ascontiguousarray(out, dtype=np.float32), res


def kernel(**inputs) -> np.ndarray:
    out, _ = run(inputs, trace=False)
    return out


# revision 11
# speedup vs baseline: 1.2050x; 1.2050x over previous
"""Trainium2 Bass kernel for AdaptiveStochasticSNN (v3: tapered windows, packed
PSUM, per-block FC2, ACT-engine bias-subtract, incremental reduce).

Model: x[B,T,D] -> FC1(D->H) -> StochasticAdaptiveLIF -> FC2(H->A)
       -> StochasticAdaptiveLIF -> mean spikes over T.   B,T,D,H,A = 256,64,6400,1000,4

Strategy (8 NeuronCores, data-parallel over batch, 32 batches/core):
- FC1 in fp16 at the full 2.4GHz PE column rate; W1 (13.1MB fp16) SBUF-resident.
- The bernoulli draw  u < sigmoid(mem - 1 - theta)  is host-transformed to
  logit(u) + 1 + theta < mem; theta tracked as psi = 20*theta - 10 so its
  update is one DVE op. lu1 stored fp16 (mixed-dtype DVE ops).
- Time processed in 8 GEMM windows of [16,8,8,8,8,8,4,4] steps. LIF blocks are
  [8,8,8,8,8,8,8,4,4] (w0 = 2 blocks). Per window w: GEMM(w) overlaps
  LIF1(blocks of w-1) on DVE; FC2(b) matmuls run right after GEMM(w) for w-1's
  blocks (LIF1 always finishes first -> no PE stall); c2 = rs2 - ps2 runs on
  the ACT engine (bias=rs2, scale=-1); LIF2(w-1) + partial spike reduction on
  DVE behind LIF1(w).
- PSUM: one FIFO tag, bufs=8 banks. w0 uses 8 full banks; later windows pack
  2 h-chunks per bank ([128, 2, ncol]) so 4 banks/window + fc2 banks rotate.
- Startup: first DMAs are single-kc slivers (x kc0, W1 kc0) so the first
  matmul issues ~2us after the preamble barrier.
"""

import sys

sys.path.insert(0, "/opt/trn_rl_repo")

import numpy as np

# ---- problem dims (hardcoded; kernel.py must be self-contained) ----
B, T, D, H, A = 256, 64, 6400, 1000, 4
HP = 1024          # H padded to 8*128
NCORES = 8
BC = B // NCORES   # 32 batches per core
BT = BC * T        # 2048 bt-columns per core, ordered bt = t*BC + b
KC = D // 128      # 50 contraction chunks
MC = HP // 128     # 8 h-chunks
WSTEPS = [16, 16, 16, 8, 4, 4]          # GEMM window timesteps
NWIN = len(WSTEPS)
WCOLS = [s * BC for s in WSTEPS]
WCOL0 = [0]
for _c in WCOLS:
    WCOL0.append(WCOL0[-1] + _c)
WSTEP0 = [0]
for _s in WSTEPS:
    WSTEP0.append(WSTEP0[-1] + _s)
# LIF blocks: 16-step windows split into two 8-step blocks
BSTEPS = [8, 8, 8, 8, 8, 8, 8, 4, 4]
NBLK = len(BSTEPS)
BSTEP0 = [0]
for _s in BSTEPS:
    BSTEP0.append(BSTEP0[-1] + _s)
BCOLS = [s * BC for s in BSTEPS]
BCOL0 = [0]
for _c in BCOLS:
    BCOL0.append(BCOL0[-1] + _c)
BLOCKS_OF_W = [[0, 1], [2, 3], [4, 5], [6], [7], [8]]
BETA = 0.9
TH_DEC = 0.9
TH_PLUS = 0.05

_CACHE = {}


def _build_graph():
    import concourse.bass as bass
    import concourse.tile as tile
    from concourse import bacc, mybir
    from concourse.alu_op_type import AluOpType as op
    from contextlib import ExitStack

    F32 = mybir.dt.float32
    F16 = mybir.dt.float16
    AF = mybir.ActivationFunctionType

    nc = bacc.Bacc("TRN2", target_bir_lowering=False, debug=False, num_devices=NCORES)

    # host-prepped layouts, all contiguous per partition:
    # xh:  [p, w, kc, col]   fp16 (col within window)
    # w1h: [p, kc, mc, m]    fp16
    # lu1: [p, blk, mc, c]   fp16 (c within block)
    xh = nc.declare_dram_parameter("xh", [128, KC * BT], F16, isOutput=False)
    w1h = nc.declare_dram_parameter("w1h", [128, KC * MC * 128], F16, isOutput=False)
    b1t = nc.declare_dram_parameter("b1t", [128, MC], F32, isOutput=False)
    lu1 = nc.declare_dram_parameter("lu1", [128, MC * BT], F16, isOutput=False)
    lu2 = nc.declare_dram_parameter("lu2", [A, T * BC], F32, isOutput=False)
    w2t = nc.declare_dram_parameter("w2t", [128, MC, A], F16, isOutput=False)
    rs2c = nc.declare_dram_parameter("rs2c", [A, 512], F32, isOutput=False)
    out = nc.declare_dram_parameter("out", [A, BC], F32, isOutput=True)

    with tile.TileContext(nc) as tc, ExitStack() as ctx:
        p_w1 = ctx.enter_context(tc.tile_pool(name="w1p", bufs=1))
        p_x = ctx.enter_context(tc.tile_pool(name="xp", bufs=3))
        p_cur = ctx.enter_context(tc.tile_pool(name="curp", bufs=4))
        p_lu = ctx.enter_context(tc.tile_pool(name="lup", bufs=4))
        p_ge = ctx.enter_context(tc.tile_pool(name="gep", bufs=2))
        p_st = ctx.enter_context(tc.tile_pool(name="stp", bufs=1))
        p_sc = ctx.enter_context(tc.tile_pool(name="scp", bufs=2))
        p_ps = ctx.enter_context(
            tc.tile_pool(name="psp", bufs=8, space=bass.MemorySpace.PSUM)
        )

        # ---- constants / states ----
        b1_sb = p_st.tile([128, MC], F32, name="b1_sb")
        nc.sync.dma_start(b1_sb[:], b1t[:])
        w2_sb = p_st.tile([128, MC, A], F16, name="w2_sb")
        nc.sync.dma_start(w2_sb[:], w2t[:])
        rs2_sb = p_st.tile([A, 512], F32, name="rs2_sb")
        nc.sync.dma_start(rs2_sb[:], rs2c[:])
        lu2_sb = p_st.tile([A, T * BC], F32, name="lu2_sb")
        nc.sync.dma_start(lu2_sb[:], lu2[:])

        w1_sb = p_w1.tile([128, KC * MC * 128], F16, name="w1_sb")

        mem = p_st.tile([128, MC, BC], F32, name="mem")
        nc.gpsimd.memset(mem[:], 0.0)
        psi = p_st.tile([128, MC, BC], F32, name="psi")
        nc.gpsimd.memset(psi[:], -10.0)
        mem2 = p_st.tile([A, BC], F32, name="mem2")
        nc.gpsimd.memset(mem2[:], 0.0)
        psi2 = p_st.tile([A, BC], F32, name="psi2")
        nc.gpsimd.memset(psi2[:], -10.0)
        sum2g = p_st.tile([A, BC], F32, name="sum2g")
        nc.gpsimd.memset(sum2g[:], 0.0)

        cur_tiles = [None] * NBLK
        ge_tiles = [None] * NBLK
        lu_tiles = [None] * NBLK
        c2_tiles = [None] * NBLK
        ge2_tiles = [None] * NBLK
        ps2_tiles = [None] * NBLK

        def emit_lu_dma(b):
            bs = BSTEPS[b]
            lu_t = p_lu.tile([128, MC, bs * BC], F16, tag="lu", name=f"lu_{b}")
            lu_tiles[b] = lu_t
            src = BCOL0[b] * MC
            nc.sync.dma_start(lu_t[:], lu1[:, src : src + MC * bs * BC])

        def emit_lif1(b):
            """LIF1 recurrence for block b on DVE."""
            bs = BSTEPS[b]
            cur1 = cur_tiles[b]
            lu_t = lu_tiles[b]
            ge_t = p_ge.tile([128, MC, bs * BC], F16, tag="ge", name=f"ge_{b}")
            ge_tiles[b] = ge_t
            for s in range(bs):
                c_sl = cur1[:, :, s * BC : (s + 1) * BC]
                mi = p_sc.tile([128, MC, BC], F32, tag="mi", name=f"mi_{b}_{s}")
                nc.vector.scalar_tensor_tensor(
                    mi[:], mem[:], BETA, c_sl, op0=op.mult, op1=op.add
                )
                lp = p_sc.tile([128, MC, BC], F32, tag="lp", name=f"lp_{b}_{s}")
                lu_sl = lu_t[:, :, s * BC : (s + 1) * BC]
                nc.vector.scalar_tensor_tensor(
                    lp[:], psi[:], TH_PLUS, lu_sl, op0=op.mult, op1=op.add
                )
                ge_sl = ge_t[:, :, s * BC : (s + 1) * BC]
                nc.vector.tensor_tensor(ge_sl, mi[:], lp[:], op.is_le)
                nc.vector.tensor_tensor(mem[:], mi[:], ge_sl, op.mult)
                nc.vector.scalar_tensor_tensor(
                    psi[:], psi[:], TH_DEC, ge_sl, op0=op.mult, op1=op.subtract
                )

        def emit_fc2_mm(b):
            """FC2 matmuls for block b: ps2[A, bcol] = W2 @ ge_b (PE)."""
            bcol = BCOLS[b]
            ge_t = ge_tiles[b]
            ps2 = p_ps.tile([A, bcol], F32, tag="bank", name=f"ps2_{b}")
            ps2_tiles[b] = ps2
            for k2 in range(MC):
                nc.tensor.matmul(
                    ps2[:],
                    w2_sb[:, k2, :],
                    ge_t[:, k2, :],
                    start=(k2 == 0),
                    stop=(k2 == MC - 1),
                )

        def emit_fc2_sub(b):
            """c2 = rs2 - ps2 on the ACT engine (bias=rs2 per-partition)."""
            bcol = BCOLS[b]
            c2 = p_sc.tile([A, bcol], F32, tag="c2", name=f"c2_{b}")
            nc.scalar.activation(
                c2[:], ps2_tiles[b][:], AF.Identity, bias=rs2_sb[:, 0:1], scale=-1.0
            )
            c2_tiles[b] = c2

        def emit_lif2(b):
            """LIF2 recurrence for block b on DVE; ge2 stored [A, BC, bs]."""
            bs = BSTEPS[b]
            ge2_t = p_sc.tile([A, BC, bs], F16, tag="ge2", name=f"ge2_{b}")
            ge2_tiles[b] = ge2_t
            c2 = c2_tiles[b]
            for s in range(bs):
                t = BSTEP0[b] + s
                cur2 = c2[:, s * BC : (s + 1) * BC]
                m2i = p_sc.tile([A, BC], F32, tag="m2i", name=f"m2i_{t}")
                nc.vector.scalar_tensor_tensor(
                    m2i[:], mem2[:], BETA, cur2, op0=op.mult, op1=op.add
                )
                lp2 = p_sc.tile([A, BC], F32, tag="lp2", name=f"lp2_{t}")
                nc.vector.scalar_tensor_tensor(
                    lp2[:],
                    psi2[:],
                    TH_PLUS,
                    lu2_sb[:, t * BC : (t + 1) * BC],
                    op0=op.mult,
                    op1=op.add,
                )
                ge2_sl = ge2_t[:, :, s]
                nc.vector.tensor_tensor(ge2_sl, m2i[:], lp2[:], op.is_le)
                nc.vector.tensor_tensor(mem2[:], m2i[:], ge2_sl, op.mult)
                nc.vector.scalar_tensor_tensor(
                    psi2[:], psi2[:], TH_DEC, ge2_sl, op0=op.mult, op1=op.subtract
                )

        def emit_reduce(b):
            """Accumulate spike counts for block b into sum2g (DVE)."""
            tmp = p_sc.tile([A, BC], F32, tag="rtmp", name=f"rtmp_{b}")
            nc.vector.tensor_reduce(
                tmp[:], ge2_tiles[b][:], mybir.AxisListType.X, op.add
            )
            nc.vector.tensor_tensor(sum2g[:], sum2g[:], tmp[:], op.add)

        # ---------------- main window loop ----------------
        for w in range(NWIN):
            ncol = WCOLS[w]
            blocks = BLOCKS_OF_W[w]
            # cur tiles for this window's blocks
            for b in blocks:
                cur_tiles[b] = p_cur.tile(
                    [128, MC, BCOLS[b]], F32, tag="cur", name=f"cur_{b}"
                )
            # psum accumulators: one full bank per h-chunk (two accumulation
            # groups must NOT share a bank: interleaved start/accumulate to
            # one bank corrupts the other group's region)
            accs = [
                p_ps.tile([128, ncol], F32, tag="bank", name=f"acc_{w}_{mc}")
                for mc in range(MC)
            ]

            def accsl(mc):
                return accs[mc][:]

            # x DMA groups + matmuls (5-kc groups: 25-kc tiles slow the PE
            # sequencer by ~20ns/matmul)
            XG = 5
            groups = [list(range(g * XG, (g + 1) * XG)) for g in range(KC // XG)]
            for gi, kcs in enumerate(groups):
                xg = p_x.tile(
                    [128, len(kcs) * ncol],
                    F16,
                    tag="x5",
                    name=f"x_{w}_{gi}",
                )
                src = WCOL0[w] * KC + kcs[0] * ncol
                if w == 0 and gi == 0:
                    # sliver DMAs: first matmul waits on only x kc0 + W1 kc0
                    nc.sync.dma_start(xg[:, :ncol], xh[:, src : src + ncol])
                    nc.sync.dma_start(
                        w1_sb[:, : MC * 128], w1h[:, : MC * 128]
                    )
                    nc.sync.dma_start(
                        xg[:, ncol:], xh[:, src + ncol : src + XG * ncol]
                    )
                    nc.sync.dma_start(
                        w1_sb[:, MC * 128 : XG * MC * 128],
                        w1h[:, MC * 128 : XG * MC * 128],
                    )
                else:
                    nc.sync.dma_start(xg[:], xh[:, src : src + len(kcs) * ncol])
                    if w == 0:
                        wsrc = gi * XG * MC * 128
                        nc.sync.dma_start(
                            w1_sb[:, wsrc : wsrc + XG * MC * 128],
                            w1h[:, wsrc : wsrc + XG * MC * 128],
                        )
                # lu prefetch for this window's blocks, spread mid-window
                for bidx, b in enumerate(blocks):
                    if gi == 4 + 3 * bidx:
                        emit_lu_dma(b)
                for i, kc in enumerate(kcs):
                    for mc in range(MC):
                        nc.tensor.matmul(
                            accsl(mc),
                            w1_sb[
                                :, (kc * MC + mc) * 128 : (kc * MC + mc + 1) * 128
                            ],
                            xg[:, i * ncol : (i + 1) * ncol],
                            start=(kc == 0),
                            stop=(kc == KC - 1),
                        )
                        if kc == KC - 1:
                            # psum -> sbuf (+b1), freeing the bank early
                            for bi, b in enumerate(blocks):
                                nc.scalar.activation(
                                    cur_tiles[b][:, mc, :],
                                    accs[mc][:, bi * 256 : bi * 256 + BCOLS[b]],
                                    AF.Identity,
                                    bias=b1_sb[:, mc : mc + 1],
                                    scale=1.0,
                                )
            # PE: FC2 for previous window's blocks (LIF1 finished during this
            # window); ACT: c2 subs right behind
            if w >= 1:
                for b in BLOCKS_OF_W[w - 1]:
                    emit_fc2_mm(b)
                for b in BLOCKS_OF_W[w - 1]:
                    emit_fc2_sub(b)
            # DVE: LIF1 for this window's blocks, then LIF2 + reduce for the
            # previous window's blocks
            for b in blocks:
                emit_lif1(b)
            if w >= 1:
                for b in BLOCKS_OF_W[w - 1]:
                    emit_lif2(b)
                    emit_reduce(b)

        # ---------- tail ----------
        bl = NBLK - 1
        emit_fc2_mm(bl)
        emit_fc2_sub(bl)
        emit_lif2(bl)
        emit_reduce(bl)

        outf = p_st.tile([A, BC], F32, name="outf")
        nc.scalar.activation(outf[:], sum2g[:], AF.Copy, bias=1.0, scale=-1.0 / T)
        nc.sync.dma_start(out[:], outf[:])

    nc.compile()
    return nc


def _host_prep(x, W1, b1, W2, b2, u1, u2):
    """Shard + lay out inputs for the 8 cores. Returns in_maps."""
    x = np.asarray(x, dtype=np.float32)
    W1 = np.asarray(W1, dtype=np.float32)
    b1 = np.asarray(b1, dtype=np.float32)
    W2 = np.asarray(W2, dtype=np.float32)
    b2 = np.asarray(b2, dtype=np.float32)

    BIG = np.float32(30000.0)
    with np.errstate(divide="ignore"):
        u1d = np.asarray(u1, dtype=np.float64)
        lu1f = np.clip(np.log(u1d / (1.0 - u1d)) + 1.5, -3e4, 3e4).astype(np.float32)
        u2d = np.asarray(u2, dtype=np.float64)
        lu2f = np.clip(np.log(u2d / (1.0 - u2d)) + 1.5, -1e30, 1e30).astype(np.float32)

    W1TP = np.zeros((D, HP), np.float32)
    W1TP[:, :H] = W1.T
    w1h = np.ascontiguousarray(
        W1TP.reshape(KC, 128, MC, 128).transpose(1, 0, 2, 3).reshape(128, KC * MC * 128)
    ).astype(np.float16)

    b1p = np.zeros((HP,), np.float32)
    b1p[:H] = b1
    b1t = np.ascontiguousarray(b1p.reshape(MC, 128).T)  # [128, MC]

    W2f16 = W2.T.astype(np.float16)  # [H, A]
    W2TP = np.zeros((HP, A), np.float16)
    W2TP[:H, :] = W2f16
    w2t = np.ascontiguousarray(W2TP.reshape(MC, 128, A).transpose(1, 0, 2))

    rs2 = (W2f16.astype(np.float64).sum(axis=0) + b2).astype(np.float32)  # [A]
    rs2c = np.ascontiguousarray(np.repeat(rs2[:, None], 512, axis=1))  # [A, 512]

    in_maps = []
    for c in range(NCORES):
        bs, be = c * BC, (c + 1) * BC
        # x: [D, bt] -> [p, w, kc, col] fp16
        xt = x[bs:be].transpose(2, 1, 0).reshape(D, BT).astype(np.float16)
        arr = xt.reshape(KC, 128, BT).transpose(1, 0, 2)  # [p, kc, bt]
        xh_c = np.concatenate(
            [
                np.ascontiguousarray(arr[:, :, WCOL0[w] : WCOL0[w + 1]]).reshape(
                    128, -1
                )
                for w in range(NWIN)
            ],
            axis=1,
        )
        xh_c = np.ascontiguousarray(xh_c)
        # lu1: [p, blk, mc, c] fp16
        lu_c = np.full((T, BC, HP), BIG, np.float32)
        lu_c[:, :, :H] = lu1f[:, bs:be, :]
        lu_c = lu_c.transpose(2, 0, 1).reshape(HP, BT)  # [h, t*BC+b]
        lu_c = lu_c.reshape(MC, 128, BT).transpose(1, 0, 2)  # [p, mc, bt]
        lu_b = np.concatenate(
            [
                np.ascontiguousarray(
                    lu_c[:, :, BCOL0[b] : BCOL0[b + 1]]
                ).reshape(128, -1)
                for b in range(NBLK)
            ],
            axis=1,
        ).astype(np.float16)
        lu_b = np.ascontiguousarray(lu_b)
        # lu2: [A, T*BC]
        lu2_c = np.ascontiguousarray(
            lu2f[:, bs:be, :].transpose(2, 0, 1).reshape(A, T * BC)
        )
        in_maps.append(
            {
                "xh": xh_c,
                "w1h": w1h,
                "b1t": b1t,
                "lu1": lu_b,
                "lu2": lu2_c,
                "w2t": w2t,
                "rs2c": rs2c,
            }
        )
    return in_maps


def run(inputs, trace=False):
    """Build (cached), run on 8 cores, gather. Returns (out, BassKernelResults)."""
    from concourse.bass_utils import run_bass_kernel_spmd

    if "nc" not in _CACHE:
        _CACHE["nc"] = _build_graph()
    nc = _CACHE["nc"]
    in_maps = _host_prep(**inputs)
    res = run_bass_kernel_spmd(nc, in_maps, core_ids=list(range(NCORES)), trace=trace)
    out = np.concatenate(
        [res.results[c]["out"].T for c in range(NCORES)], axis=0
    )
    return np.ascontiguousarray(out, dtype=np.float32), res


def kernel(**inputs) -> np.ndarray:
    out, _ = run(inputs, trace=False)
    return out


# revision 16
# speedup vs baseline: 1.2087x; 1.0031x over previous
"""Trainium2 Bass kernel for AdaptiveStochasticSNN (v3: tapered windows, packed
PSUM, per-block FC2, ACT-engine bias-subtract, incremental reduce).

Model: x[B,T,D] -> FC1(D->H) -> StochasticAdaptiveLIF -> FC2(H->A)
       -> StochasticAdaptiveLIF -> mean spikes over T.   B,T,D,H,A = 256,64,6400,1000,4

Strategy (8 NeuronCores, data-parallel over batch, 32 batches/core):
- FC1 in fp16 at the full 2.4GHz PE column rate; W1 (13.1MB fp16) SBUF-resident.
- The bernoulli draw  u < sigmoid(mem - 1 - theta)  is host-transformed to
  logit(u) + 1 + theta < mem; theta tracked as psi = 20*theta - 10 so its
  update is one DVE op. lu1 stored fp16 (mixed-dtype DVE ops).
- Time processed in 8 GEMM windows of [16,8,8,8,8,8,4,4] steps. LIF blocks are
  [8,8,8,8,8,8,8,4,4] (w0 = 2 blocks). Per window w: GEMM(w) overlaps
  LIF1(blocks of w-1) on DVE; FC2(b) matmuls run right after GEMM(w) for w-1's
  blocks (LIF1 always finishes first -> no PE stall); c2 = rs2 - ps2 runs on
  the ACT engine (bias=rs2, scale=-1); LIF2(w-1) + partial spike reduction on
  DVE behind LIF1(w).
- PSUM: one FIFO tag, bufs=8 banks. w0 uses 8 full banks; later windows pack
  2 h-chunks per bank ([128, 2, ncol]) so 4 banks/window + fc2 banks rotate.
- Startup: first DMAs are single-kc slivers (x kc0, W1 kc0) so the first
  matmul issues ~2us after the preamble barrier.
"""

import sys

sys.path.insert(0, "/opt/trn_rl_repo")

import numpy as np

# ---- problem dims (hardcoded; kernel.py must be self-contained) ----
B, T, D, H, A = 256, 64, 6400, 1000, 4
HP = 1024          # H padded to 8*128
NCORES = 8
BC = B // NCORES   # 32 batches per core
BT = BC * T        # 2048 bt-columns per core, ordered bt = t*BC + b
KC = D // 128      # 50 contraction chunks
MC = HP // 128     # 8 h-chunks
WSTEPS = [16, 16, 16, 8, 4, 4]          # GEMM window timesteps
NWIN = len(WSTEPS)
WCOLS = [s * BC for s in WSTEPS]
WCOL0 = [0]
for _c in WCOLS:
    WCOL0.append(WCOL0[-1] + _c)
WSTEP0 = [0]
for _s in WSTEPS:
    WSTEP0.append(WSTEP0[-1] + _s)
# LIF blocks: 16-step windows split into two 8-step blocks
BSTEPS = [8, 8, 8, 8, 8, 8, 8, 4, 4]
NBLK = len(BSTEPS)
BSTEP0 = [0]
for _s in BSTEPS:
    BSTEP0.append(BSTEP0[-1] + _s)
BCOLS = [s * BC for s in BSTEPS]
BCOL0 = [0]
for _c in BCOLS:
    BCOL0.append(BCOL0[-1] + _c)
BLOCKS_OF_W = [[0, 1], [2, 3], [4, 5], [6], [7], [8]]
BETA = 0.9
TH_DEC = 0.9
TH_PLUS = 0.05

_CACHE = {}


def _build_graph():
    import concourse.bass as bass
    import concourse.tile as tile
    from concourse import bacc, mybir
    from concourse.alu_op_type import AluOpType as op
    from contextlib import ExitStack

    F32 = mybir.dt.float32
    F16 = mybir.dt.float16
    AF = mybir.ActivationFunctionType

    nc = bacc.Bacc("TRN2", target_bir_lowering=False, debug=False, num_devices=NCORES)

    # host-prepped layouts, all contiguous per partition:
    # xh:  [p, w, kc, col]   fp16 (col within window)
    # w1h: [p, kc, mc, m]    fp16
    # lu1: [p, blk, mc, c]   fp16 (c within block)
    xh = nc.declare_dram_parameter("xh", [128, KC * BT], F16, isOutput=False)
    w1h = nc.declare_dram_parameter("w1h", [128, KC * MC * 128], F16, isOutput=False)
    b1t = nc.declare_dram_parameter("b1t", [128, MC], F32, isOutput=False)
    lu1 = nc.declare_dram_parameter("lu1", [128, MC * BT], F16, isOutput=False)
    lu2 = nc.declare_dram_parameter("lu2", [A, T * BC], F32, isOutput=False)
    w2t = nc.declare_dram_parameter("w2t", [128, MC, A], F16, isOutput=False)
    rs2c = nc.declare_dram_parameter("rs2c", [A, 512], F32, isOutput=False)
    out = nc.declare_dram_parameter("out", [A, BC], F32, isOutput=True)

    with tile.TileContext(nc) as tc, ExitStack() as ctx:
        p_w1 = ctx.enter_context(tc.tile_pool(name="w1p", bufs=1))
        p_x = ctx.enter_context(tc.tile_pool(name="xp", bufs=3))
        p_cur = ctx.enter_context(tc.tile_pool(name="curp", bufs=4))
        p_lu = ctx.enter_context(tc.tile_pool(name="lup", bufs=4))
        p_ge = ctx.enter_context(tc.tile_pool(name="gep", bufs=2))
        p_st = ctx.enter_context(tc.tile_pool(name="stp", bufs=1))
        p_sc = ctx.enter_context(tc.tile_pool(name="scp", bufs=2))
        p_ps = ctx.enter_context(
            tc.tile_pool(name="psp", bufs=8, space=bass.MemorySpace.PSUM)
        )

        # ---- constants / states (DMA triggers deferred past the first x/W1
        # slivers so the first matmul isn't queued behind them on Sync) ----
        b1_sb = p_st.tile([128, MC], F32, name="b1_sb")
        w2_sb = p_st.tile([128, MC, A], F16, name="w2_sb")
        rs2_sb = p_st.tile([A, 512], F32, name="rs2_sb")
        lu2_sb = p_st.tile([A, T * BC], F32, name="lu2_sb")

        def emit_const_dmas():
            nc.sync.dma_start(b1_sb[:], b1t[:])
            nc.sync.dma_start(w2_sb[:], w2t[:])
            nc.sync.dma_start(rs2_sb[:], rs2c[:])
            nc.sync.dma_start(lu2_sb[:], lu2[:])

        w1_sb = p_w1.tile([128, KC * MC * 128], F16, name="w1_sb")

        mem = p_st.tile([128, MC, BC], F32, name="mem")
        nc.gpsimd.memset(mem[:], 0.0)
        psi = p_st.tile([128, MC, BC], F32, name="psi")
        nc.gpsimd.memset(psi[:], -10.0)
        mem2 = p_st.tile([A, BC], F32, name="mem2")
        nc.gpsimd.memset(mem2[:], 0.0)
        psi2 = p_st.tile([A, BC], F32, name="psi2")
        nc.gpsimd.memset(psi2[:], -10.0)
        sum2g = p_st.tile([A, BC], F32, name="sum2g")
        nc.gpsimd.memset(sum2g[:], 0.0)

        cur_tiles = [None] * NBLK
        ge_tiles = [None] * NBLK
        lu_tiles = [None] * NBLK
        c2_tiles = [None] * NBLK
        ge2_tiles = [None] * NBLK
        ps2_tiles = [None] * NBLK

        def emit_lu_dma(b):
            bs = BSTEPS[b]
            lu_t = p_lu.tile([128, MC, bs * BC], F16, tag="lu", name=f"lu_{b}")
            lu_tiles[b] = lu_t
            src = BCOL0[b] * MC
            nc.sync.dma_start(lu_t[:], lu1[:, src : src + MC * bs * BC])

        def emit_lif1(b, mem_t=None, psi_t=None, f16=False):
            """LIF1 recurrence for block b on DVE (all-fp16 for the tail block:
            2x DVE rate; precision impact verified negligible for 4 steps)."""
            bs = BSTEPS[b]
            cur1 = cur_tiles[b]
            lu_t = lu_tiles[b]
            mem_t = mem if mem_t is None else mem_t
            psi_t = psi if psi_t is None else psi_t
            DT = F16 if f16 else F32
            tg = "16" if f16 else ""
            ge_t = p_ge.tile([128, MC, bs * BC], F16, tag="ge", name=f"ge_{b}")
            ge_tiles[b] = ge_t
            for s in range(bs):
                c_sl = cur1[:, :, s * BC : (s + 1) * BC]
                mi = p_sc.tile([128, MC, BC], DT, tag="mi" + tg, name=f"mi_{b}_{s}")
                nc.vector.scalar_tensor_tensor(
                    mi[:], mem_t[:], BETA, c_sl, op0=op.mult, op1=op.add
                )
                lp = p_sc.tile([128, MC, BC], DT, tag="lp" + tg, name=f"lp_{b}_{s}")
                lu_sl = lu_t[:, :, s * BC : (s + 1) * BC]
                nc.vector.scalar_tensor_tensor(
                    lp[:], psi_t[:], TH_PLUS, lu_sl, op0=op.mult, op1=op.add
                )
                ge_sl = ge_t[:, :, s * BC : (s + 1) * BC]
                nc.vector.tensor_tensor(ge_sl, mi[:], lp[:], op.is_le)
                nc.vector.tensor_tensor(mem_t[:], mi[:], ge_sl, op.mult)
                nc.vector.scalar_tensor_tensor(
                    psi_t[:], psi_t[:], TH_DEC, ge_sl, op0=op.mult, op1=op.subtract
                )

        def emit_fc2_mm(b):
            """FC2 matmuls for block b: ps2[A, bcol] = W2 @ ge_b (PE)."""
            bcol = BCOLS[b]
            ge_t = ge_tiles[b]
            ps2 = p_ps.tile([A, bcol], F32, tag="bank", name=f"ps2_{b}")
            ps2_tiles[b] = ps2
            for k2 in range(MC):
                nc.tensor.matmul(
                    ps2[:],
                    w2_sb[:, k2, :],
                    ge_t[:, k2, :],
                    start=(k2 == 0),
                    stop=(k2 == MC - 1),
                )

        def emit_fc2_sub(b):
            """c2 = rs2 - ps2 on the ACT engine (bias=rs2 per-partition)."""
            bcol = BCOLS[b]
            c2 = p_sc.tile([A, bcol], F32, tag="c2", name=f"c2_{b}")
            nc.scalar.activation(
                c2[:], ps2_tiles[b][:], AF.Identity, bias=rs2_sb[:, 0:1], scale=-1.0
            )
            c2_tiles[b] = c2

        def emit_lif2(b):
            """LIF2 recurrence for block b on DVE; ge2 stored [A, BC, bs]."""
            bs = BSTEPS[b]
            ge2_t = p_sc.tile([A, BC, bs], F16, tag="ge2", name=f"ge2_{b}")
            ge2_tiles[b] = ge2_t
            c2 = c2_tiles[b]
            for s in range(bs):
                t = BSTEP0[b] + s
                cur2 = c2[:, s * BC : (s + 1) * BC]
                m2i = p_sc.tile([A, BC], F32, tag="m2i", name=f"m2i_{t}")
                nc.vector.scalar_tensor_tensor(
                    m2i[:], mem2[:], BETA, cur2, op0=op.mult, op1=op.add
                )
                lp2 = p_sc.tile([A, BC], F32, tag="lp2", name=f"lp2_{t}")
                nc.vector.scalar_tensor_tensor(
                    lp2[:],
                    psi2[:],
                    TH_PLUS,
                    lu2_sb[:, t * BC : (t + 1) * BC],
                    op0=op.mult,
                    op1=op.add,
                )
                ge2_sl = ge2_t[:, :, s]
                nc.vector.tensor_tensor(ge2_sl, m2i[:], lp2[:], op.is_le)
                nc.vector.tensor_tensor(mem2[:], m2i[:], ge2_sl, op.mult)
                nc.vector.scalar_tensor_tensor(
                    psi2[:], psi2[:], TH_DEC, ge2_sl, op0=op.mult, op1=op.subtract
                )

        def emit_reduce(b):
            """Accumulate spike counts for block b into sum2g (DVE)."""
            tmp = p_sc.tile([A, BC], F32, tag="rtmp", name=f"rtmp_{b}")
            nc.vector.tensor_reduce(
                tmp[:], ge2_tiles[b][:], mybir.AxisListType.X, op.add
            )
            nc.vector.tensor_tensor(sum2g[:], sum2g[:], tmp[:], op.add)

        # ---------------- main window loop ----------------
        for w in range(NWIN):
            ncol = WCOLS[w]
            blocks = BLOCKS_OF_W[w]
            # cur tiles for this window's blocks
            for b in blocks:
                cur_tiles[b] = p_cur.tile(
                    [128, MC, BCOLS[b]],
                    F16 if b == NBLK - 1 else F32,
                    tag="cur",
                    name=f"cur_{b}",
                )
            # psum accumulators: one full bank per h-chunk (two accumulation
            # groups must NOT share a bank: interleaved start/accumulate to
            # one bank corrupts the other group's region)
            accs = [
                p_ps.tile([128, ncol], F32, tag="bank", name=f"acc_{w}_{mc}")
                for mc in range(MC)
            ]

            def accsl(mc):
                return accs[mc][:]

            # x DMA groups + matmuls (5-kc groups: 25-kc tiles slow the PE
            # sequencer by ~20ns/matmul)
            XG = 5
            groups = [list(range(g * XG, (g + 1) * XG)) for g in range(KC // XG)]
            for gi, kcs in enumerate(groups):
                xg = p_x.tile(
                    [128, len(kcs) * ncol],
                    F16,
                    tag="x5",
                    name=f"x_{w}_{gi}",
                )
                src = WCOL0[w] * KC + kcs[0] * ncol
                if w == 0 and gi == 0:
                    # sliver DMAs: first matmul waits on only x kc0 + W1 kc0
                    nc.sync.dma_start(xg[:, :ncol], xh[:, src : src + ncol])
                    nc.sync.dma_start(
                        w1_sb[:, : MC * 128], w1h[:, : MC * 128]
                    )
                    nc.sync.dma_start(
                        xg[:, ncol:], xh[:, src + ncol : src + XG * ncol]
                    )
                    nc.sync.dma_start(
                        w1_sb[:, MC * 128 : XG * MC * 128],
                        w1h[:, MC * 128 : XG * MC * 128],
                    )
                    emit_const_dmas()
                else:
                    nc.sync.dma_start(xg[:], xh[:, src : src + len(kcs) * ncol])
                    if w == 0:
                        wsrc = gi * XG * MC * 128
                        nc.sync.dma_start(
                            w1_sb[:, wsrc : wsrc + XG * MC * 128],
                            w1h[:, wsrc : wsrc + XG * MC * 128],
                        )
                # lu prefetch for this window's blocks, spread mid-window
                for bidx, b in enumerate(blocks):
                    if gi == 4 + 3 * bidx:
                        emit_lu_dma(b)
                for i, kc in enumerate(kcs):
                    for mc in range(MC):
                        nc.tensor.matmul(
                            accsl(mc),
                            w1_sb[
                                :, (kc * MC + mc) * 128 : (kc * MC + mc + 1) * 128
                            ],
                            xg[:, i * ncol : (i + 1) * ncol],
                            start=(kc == 0),
                            stop=(kc == KC - 1),
                        )
                        if kc == KC - 1:
                            # psum -> sbuf (+b1), freeing the bank early
                            for bi, b in enumerate(blocks):
                                nc.scalar.activation(
                                    cur_tiles[b][:, mc, :],
                                    accs[mc][:, bi * 256 : bi * 256 + BCOLS[b]],
                                    AF.Identity,
                                    bias=b1_sb[:, mc : mc + 1],
                                    scale=1.0,
                                )
            # PE: FC2 for previous window's blocks (LIF1 finished during this
            # window); ACT: c2 subs right behind
            if w >= 1:
                for b in BLOCKS_OF_W[w - 1]:
                    emit_fc2_mm(b)
                for b in BLOCKS_OF_W[w - 1]:
                    emit_fc2_sub(b)
            # DVE: LIF1 for this window's blocks, then LIF2 + reduce for the
            # previous window's blocks
            for b in blocks:
                if b == NBLK - 1:
                    # tail block runs LIF1 in fp16: convert state (on ACT,
                    # ordered after LIF1(b-1) via tile deps)
                    mem16 = p_st.tile([128, MC, BC], F16, name="mem16")
                    nc.scalar.activation(
                        mem16[:], mem[:], AF.Copy, bias=0.0, scale=1.0
                    )
                    psi16 = p_st.tile([128, MC, BC], F16, name="psi16")
                    nc.scalar.activation(
                        psi16[:], psi[:], AF.Copy, bias=0.0, scale=1.0
                    )
                    emit_lif1(b, mem16, psi16, f16=True)
                else:
                    emit_lif1(b)
            if w >= 1:
                for b in BLOCKS_OF_W[w - 1]:
                    emit_lif2(b)
                    emit_reduce(b)

        # ---------- tail ----------
        bl = NBLK - 1
        emit_fc2_mm(bl)
        emit_fc2_sub(bl)
        emit_lif2(bl)
        emit_reduce(bl)

        outf = p_st.tile([A, BC], F32, name="outf")
        nc.scalar.activation(outf[:], sum2g[:], AF.Copy, bias=1.0, scale=-1.0 / T)
        nc.sync.dma_start(out[:], outf[:])

    nc.compile()
    return nc


def _host_prep(x, W1, b1, W2, b2, u1, u2):
    """Shard + lay out inputs for the 8 cores. Returns in_maps."""
    x = np.asarray(x, dtype=np.float32)
    W1 = np.asarray(W1, dtype=np.float32)
    b1 = np.asarray(b1, dtype=np.float32)
    W2 = np.asarray(W2, dtype=np.float32)
    b2 = np.asarray(b2, dtype=np.float32)

    BIG = np.float32(30000.0)
    with np.errstate(divide="ignore"):
        u1d = np.asarray(u1, dtype=np.float64)
        lu1f = np.clip(np.log(u1d / (1.0 - u1d)) + 1.5, -3e4, 3e4).astype(np.float32)
        u2d = np.asarray(u2, dtype=np.float64)
        lu2f = np.clip(np.log(u2d / (1.0 - u2d)) + 1.5, -1e30, 1e30).astype(np.float32)

    W1TP = np.zeros((D, HP), np.float32)
    W1TP[:, :H] = W1.T
    w1h = np.ascontiguousarray(
        W1TP.reshape(KC, 128, MC, 128).transpose(1, 0, 2, 3).reshape(128, KC * MC * 128)
    ).astype(np.float16)

    b1p = np.zeros((HP,), np.float32)
    b1p[:H] = b1
    b1t = np.ascontiguousarray(b1p.reshape(MC, 128).T)  # [128, MC]

    W2f16 = W2.T.astype(np.float16)  # [H, A]
    W2TP = np.zeros((HP, A), np.float16)
    W2TP[:H, :] = W2f16
    w2t = np.ascontiguousarray(W2TP.reshape(MC, 128, A).transpose(1, 0, 2))

    rs2 = (W2f16.astype(np.float64).sum(axis=0) + b2).astype(np.float32)  # [A]
    rs2c = np.ascontiguousarray(np.repeat(rs2[:, None], 512, axis=1))  # [A, 512]

    in_maps = []
    for c in range(NCORES):
        bs, be = c * BC, (c + 1) * BC
        # x: [D, bt] -> [p, w, kc, col] fp16
        xt = x[bs:be].transpose(2, 1, 0).reshape(D, BT).astype(np.float16)
        arr = xt.reshape(KC, 128, BT).transpose(1, 0, 2)  # [p, kc, bt]
        xh_c = np.concatenate(
            [
                np.ascontiguousarray(arr[:, :, WCOL0[w] : WCOL0[w + 1]]).reshape(
                    128, -1
                )
                for w in range(NWIN)
            ],
            axis=1,
        )
        xh_c = np.ascontiguousarray(xh_c)
        # lu1: [p, blk, mc, c] fp16
        lu_c = np.full((T, BC, HP), BIG, np.float32)
        lu_c[:, :, :H] = lu1f[:, bs:be, :]
        lu_c = lu_c.transpose(2, 0, 1).reshape(HP, BT)  # [h, t*BC+b]
        lu_c = lu_c.reshape(MC, 128, BT).transpose(1, 0, 2)  # [p, mc, bt]
        lu_b = np.concatenate(
            [
                np.ascontiguousarray(
                    lu_c[:, :, BCOL0[b] : BCOL0[b + 1]]
                ).reshape(128, -1)
                for b in range(NBLK)
            ],
            axis=1,
        ).astype(np.float16)
        lu_b = np.ascontiguousarray(lu_b)
        # lu2: [A, T*BC]
        lu2_c = np.ascontiguousarray(
            lu2f[:, bs:be, :].transpose(2, 0, 1).reshape(A, T * BC)
        )
        in_maps.append(
            {
                "xh": xh_c,
                "w1h": w1h,
                "b1t": b1t,
                "lu1": lu_b,
                "lu2": lu2_c,
                "w2t": w2t,
                "rs2c": rs2c,
            }
        )
    return in_maps


def run(inputs, trace=False):
    """Build (cached), run on 8 cores, gather. Returns (out, BassKernelResults)."""
    from concourse.bass_utils import run_bass_kernel_spmd

    if "nc" not in _CACHE:
        _CACHE["nc"] = _build_graph()
    nc = _CACHE["nc"]
    in_maps = _host_prep(**inputs)
    res = run_bass_kernel_spmd(nc, in_maps, core_ids=list(range(NCORES)), trace=trace)
    out = np.concatenate(
        [res.results[c]["out"].T for c in range(NCORES)], axis=0
    )
    return np.ascontiguousarray(out, dtype=np.float32), res


def kernel(**inputs) -> np.ndarray:
    out, _ = run(inputs, trace=False)
    return out
